# revision 11
# baseline (speedup 1.0000x reference)
"""GQA (ragged_sequence) Trainium2 kernel, 8-core tensor-parallel.

Sharding: heads/KV-groups across the 8 cores (4 query heads + 1 KV group per
core; q_w/o_w split on head axis, k_w/v_w on group axis). Batch stays whole on
every core. o_w is row-split, so each core emits a partial output summed on
host. KV caches shard naturally with the group axis.

Per-core dataflow (all layouts chosen so no on-device transposes are needed
except a tiny PE-transpose for V):
  - qk path in float32r (tf32-ish, ~bf16 speed at N>=512, 16x better accuracy)
  - scores computed transposed: scoresT[s, t] = (k @ qT), softmax sums via
    running bf16 accumulators + a ones-matmul, normalization via a K=1
    broadcast matmul.
  - ctx computed transposed: ctxT[hd, t] = sum_s v[s,hd]^T p[s,t], which feeds
    the o-projection directly (contraction dim on partitions).
"""

import os
import sys
import math

for _p in ("/opt/trn_rl_repo",):
    if _p not in sys.path and os.path.isdir(_p):
        sys.path.insert(0, _p)

import numpy as np
import ml_dtypes
from contextlib import ExitStack

import concourse.bass as bass
import concourse.tile as tile
from concourse import bacc, mybir
from concourse.bass_utils import run_bass_kernel_spmd
from concourse.masks import make_identity

BF16 = ml_dtypes.bfloat16

B, T, D = 4, 512, 4096
H, G, HD = 32, 8, 128
GS = H // G
CACHE = 3584
S = CACHE + T
NCORES = 8
HPC = H // NCORES          # heads per core = 4
KT = D // 128              # 32 contraction tiles
ST = S // 128              # 32 s-tiles
NCT = CACHE // 128         # 28 cache s-tiles
TB = T // 128              # 4 token blocks per batch
OCH = D // 512             # 8 output chunks
SOFTMAX_SCALE = 1.0 / math.sqrt(HD)

_PROGRAM_CACHE = {}
LAST_EXEC_TIME_NS = None
LAST_RESULT = None


def _build_program(act_sets):
    """Build the SPMD Bass program. act_sets: tuple per batch of the s-tile
    indices where the attention mask has any True (mask applied only there)."""
    f32 = mybir.dt.float32
    f32r = mybir.dt.float32r
    bf16 = mybir.dt.bfloat16

    nc = bacc.Bacc("TRN2", target_bir_lowering=False, debug=False,
                   num_devices=NCORES)

    # ---- DRAM I/O (per-core shards) ----
    xT_d = nc.dram_tensor("xT", [D, B * T], f32r, kind="ExternalInput")
    qw_d = nc.dram_tensor("qw", [D, HPC * HD], f32r, kind="ExternalInput")
    kw_d = nc.dram_tensor("kw", [D, HD], f32r, kind="ExternalInput")
    vw_d = nc.dram_tensor("vw", [D, HD], f32r, kind="ExternalInput")
    ow_d = nc.dram_tensor("ow", [HPC * HD, D], bf16, kind="ExternalInput")
    pkT_d = nc.dram_tensor("pkT", [B, HD, CACHE], f32r, kind="ExternalInput")
    pv_d = nc.dram_tensor("pv", [B, CACHE, HD], bf16, kind="ExternalInput")
    cosT_d = nc.dram_tensor("cosT", [B, HD, T], f32, kind="ExternalInput")
    sinT_d = nc.dram_tensor("sinT", [B, HD, T], f32, kind="ExternalInput")
    ones1_d = nc.dram_tensor("ones1", [1, 128], mybir.dt.float32r, kind="ExternalInput")
    n_mask = max(1, sum(len(a) for a in act_sets))
    mask_d = nc.dram_tensor("maskm", [n_mask, 128, 2 * T], bf16, kind="ExternalInput")
    mask_idx = {}
    mi = 0
    for b in range(B):
        for j in act_sets[b]:
            mask_idx[(b, j)] = mi
            mi += 1

    out_d = nc.dram_tensor("out_p", [B * T, D], bf16, kind="ExternalOutput")
    knewT_d = nc.dram_tensor("knewT", [B, HD, T], f32, kind="ExternalOutput")
    vnewT_d = nc.dram_tensor("vnewT", [B, HD, T], f32, kind="ExternalOutput")

    with tile.TileContext(nc) as tc, ExitStack() as ctx:
        P = lambda name, bufs, space="SBUF": ctx.enter_context(
            tc.tile_pool(name=name, bufs=bufs, space=space))

        # NOTE: tile tags default to the tile *name*; each tag gets its own
        # `bufs` slots in the pool. PSUM tags are explicit and budgeted to the
        # 8 banks: qacc(3) + mm(3) + ctxps(1) + small(1).
        p_const = P("const", 1)
        p_kw = P("kw", 1)
        p_vw = P("vw", 1)
        p_ow = P("ow", 1)
        p_xt = P("xt", 3)
        p_qw = P("qws", 4)
        p_tab = P("tab", 2)
        p_mask = P("mask", 2)
        p_kT = P("kT", 1)
        p_vall = P("vall", 1)
        p_qT = P("qT", 2)
        p_p = P("pp", 3)
        p_acc = P("acc", 2)
        p_ctx = P("ctx", 2)
        p_tmp = P("tmp", 2)
        p_stage = P("stage", 2)
        p_bc = P("bc", 2)
        p_recip = P("recip", 1)
        p_out = P("outs", 2)
        ps = P("ps", 1, space="PSUM")

        # ---- constants ----
        ident = p_const.tile([128, 128], bf16)
        make_identity(nc, ident)
        onesb = p_const.tile([128, 1], bf16)
        nc.vector.memset(onesb, 1.0)
        ones1 = p_const.tile([1, 128], f32r)
        nc.sync.dma_start(ones1, ones1_d.ap())

        # ---- resident weights ----
        kw_t = p_kw.tile([128, KT, HD], f32r)
        nc.sync.dma_start(kw_t, kw_d.ap().rearrange("(kt p) n -> p kt n", p=128))
        vw_t = p_vw.tile([128, KT, HD], f32r)
        nc.sync.dma_start(vw_t, vw_d.ap().rearrange("(kt p) n -> p kt n", p=128))
        ow_tiles = []

        # PSUM layout: tag "big" = 3 slots of [128,1024] f32 (2 banks each) for
        # projection accumulator-pairs / paired score tiles / transients, plus
        # tag "ctx2" = 1 slot [128,1024] holding both heads' ctx accumulators.
        def big(name, shape=(128, 1024), dtype=f32):
            return ps.tile(list(shape), dtype, name=name, tag="big", bufs=3)

        for b in range(B):
            bt = slice(b * T, (b + 1) * T)

            # ======== Phase A: q/k/v projections for batch b (one pass) ========
            # 6 accumulators packed two-per-[128,1024] tile (each half = 1 bank)
            ab_q01 = big("abq01")
            ab_q23 = big("abq23")
            ab_kv = big("abkv")
            halves = [ab_q01[:, 0:512], ab_q01[:, 512:1024],
                      ab_q23[:, 0:512], ab_q23[:, 512:1024],
                      ab_kv[:, 0:512], ab_kv[:, 512:1024]]
            for k in range(KT):
                xt = p_xt.tile([128, T], f32r, name="xt")
                nc.sync.dma_start(xt, xT_d.ap()[k * 128:(k + 1) * 128, bt])
                qwk = p_qw.tile([128, HPC * HD], f32r, name="qwk")
                nc.sync.dma_start(qwk, qw_d.ap()[k * 128:(k + 1) * 128, :])
                first, last = (k == 0), (k == KT - 1)
                for m in range(HPC):
                    nc.tensor.matmul(halves[m], qwk[:, m * 128:(m + 1) * 128], xt,
                                     start=first, stop=last)
                nc.tensor.matmul(halves[4], kw_t[:, k, :], xt, start=first, stop=last)
                nc.tensor.matmul(halves[5], vw_t[:, k, :], xt, start=first, stop=last)

            # ======== Phase A2: RoPE + evacuations ========
            cos_b = p_tab.tile([128, T], f32, name="cosb")
            nc.sync.dma_start(cos_b, cosT_d.ap()[b])
            sin_b = p_tab.tile([128, T], f32, name="sinb")
            nc.sync.dma_start(sin_b, sinT_d.ap()[b])
            kT_b = p_kT.tile([128, S], f32r, name="kTb")
            nc.sync.dma_start(kT_b[:, 0:CACHE], pkT_d.ap()[b])
            v_b = p_vall.tile([128, S], bf16, name="vb")
            nc.sync.dma_start(
                v_b[:, 0:CACHE].rearrange("p (j h) -> p j h", h=HD),
                pv_d.ap()[b].rearrange("(j p) h -> p j h", p=128))

            def rope(dst, src):
                # dst = src*cos + rot(src)*sin ; dst[:64] uses -src[64:]*sin
                t1 = p_tmp.tile([128, T], f32, name="t1")
                nc.vector.tensor_mul(t1, src, cos_b)
                t2 = p_tmp.tile([128, T], f32, name="t2")
                nc.vector.tensor_mul(t2[0:64, :], src[64:128, :], sin_b[0:64, :])
                nc.vector.tensor_mul(t2[64:128, :], src[0:64, :], sin_b[64:128, :])
                nc.vector.tensor_sub(dst[0:64, :], t1[0:64, :], t2[0:64, :])
                nc.vector.tensor_add(dst[64:128, :], t1[64:128, :], t2[64:128, :])

            qT_b = []
            for m in range(HPC):
                qt = p_qT.tile([128, T], f32r, name=f"qT{m}")
                rope(qt, halves[m])
                qT_b.append(qt)
            rope(kT_b[:, CACHE:S], halves[4])
            nc.sync.dma_start(knewT_d.ap()[b],
                              kT_b[:, CACHE:S].bitcast(mybir.dt.float32))
            # v: evacuate vT (f32 out + bf16), transpose to [s, hd]
            vst = p_stage.tile([128, T], f32, name="vst")
            nc.scalar.copy(vst, halves[5])
            nc.sync.dma_start(vnewT_d.ap()[b], vst)
            vT_sb = p_stage.tile([128, T], bf16, name="vTsb")
            nc.vector.tensor_copy(vT_sb, halves[5])
            for tb in range(TB):
                ps_tr = big("pstr", (128, 128), bf16)
                nc.tensor.transpose(ps_tr, vT_sb[:, tb * 128:(tb + 1) * 128], ident)
                nc.vector.tensor_copy(
                    v_b[:, CACHE + tb * 128:CACHE + (tb + 1) * 128], ps_tr)

            if b == 0:
                # deferred: o_w not needed until Phase C of b=0
                for h in range(HPC):
                    owh = p_ow.tile([128, D], bf16, name=f"ow{h}")
                    nc.sync.dma_start(owh, ow_d.ap()[h * 128:(h + 1) * 128, :])
                    ow_tiles.append(owh)

            # ======== Phase B: attention, heads paired two-per-exp ========
            ctx_sb = []
            for pr in range(HPC // 2):
                h0 = 2 * pr
                ctx2 = ps.tile([128, 1024], f32, name="ctx2", tag="ctx2", bufs=1)
                accs = [p_acc.tile([128, 1024], bf16, name=f"a{i}") for i in range(4)]
                for j in range(ST):
                    sc2 = big("sc2")
                    for hh in range(2):
                        nc.tensor.matmul(sc2[:, hh * 512:(hh + 1) * 512],
                                         kT_b[:, j * 128:(j + 1) * 128],
                                         qT_b[h0 + hh], start=True, stop=True)
                    p2 = p_p.tile([128, 1024], bf16, name="p2")
                    nc.scalar.activation(p2, sc2, mybir.ActivationFunctionType.Exp,
                                         scale=SOFTMAX_SCALE)
                    if (b, j) in mask_idx:
                        mt = p_mask.tile([128, 1024], bf16, name="mt")
                        nc.sync.dma_start(mt, mask_d.ap()[mask_idx[(b, j)]])
                        nc.vector.tensor_mul(p2, p2, mt)
                    ia = j % 4
                    if j < 4:
                        nc.vector.tensor_copy(accs[ia], p2)
                    else:
                        nc.vector.tensor_add(accs[ia], accs[ia], p2)
                    for hh in range(2):
                        nc.tensor.matmul(ctx2[:, hh * 512:(hh + 1) * 512],
                                         v_b[:, j * 128:(j + 1) * 128],
                                         p2[:, hh * 512:(hh + 1) * 512],
                                         start=(j == 0), stop=(j == ST - 1))
                nc.vector.tensor_add(accs[0], accs[0], accs[1])
                nc.vector.tensor_add(accs[2], accs[2], accs[3])
                nc.vector.tensor_add(accs[0], accs[0], accs[2])
                for hh in range(2):
                    sums_ps = big("sums", (1, T))
                    nc.tensor.matmul(sums_ps, onesb,
                                     accs[0][:, hh * 512:(hh + 1) * 512],
                                     start=True, stop=True)
                    recip32 = p_recip.tile([1, T], f32, name="recip32")
                    rscr = p_recip.tile([1, T], f32, name="rscr")
                    nc.vector.reciprocal_approx_accurate(recip32, sums_ps, rscr)
                    recip = p_recip.tile([1, T], f32r, name="recip")
                    nc.vector.tensor_copy(recip, recip32)
                    bc_ps = big("bcps", (128, T))
                    nc.tensor.matmul(bc_ps, ones1, recip, start=True, stop=True)
                    bc_sb = p_bc.tile([128, T], f32, name="bcsb")
                    nc.vector.tensor_copy(bc_sb, bc_ps)
                    cx = p_ctx.tile([128, T], bf16, name=f"cx{h0 + hh}")
                    nc.vector.tensor_mul(cx, ctx2[:, hh * 512:(hh + 1) * 512], bc_sb)
                    ctx_sb.append(cx)

            # ======== Phase C: partial o-projection for batch b ========
            for tb in range(TB):
                for chk in range(OCH):
                    ops_t = big("ops", (128, 512))
                    for h in range(HPC):
                        nc.tensor.matmul(
                            ops_t, ctx_sb[h][:, tb * 128:(tb + 1) * 128],
                            ow_tiles[h][:, chk * 512:(chk + 1) * 512],
                            start=(h == 0), stop=(h == HPC - 1))
                    o_sb = p_out.tile([128, 512], bf16, name="osb")
                    nc.scalar.copy(o_sb, ops_t)
                    nc.sync.dma_start(
                        out_d.ap()[b * T + tb * 128:b * T + (tb + 1) * 128,
                                   chk * 512:(chk + 1) * 512], o_sb)

    nc.compile()
    return nc


def kernel(x, mask, cos, sin, start_positions, prev_k, prev_v, q_w, k_w, v_w, o_w):
    global LAST_EXEC_TIME_NS
    x = np.asarray(x, dtype=np.float32)
    mask = np.asarray(mask)
    cos = np.asarray(cos, dtype=np.float32)
    sin = np.asarray(sin, dtype=np.float32)
    start_positions = np.asarray(start_positions)
    prev_k = np.asarray(prev_k, dtype=np.float32)
    prev_v = np.asarray(prev_v, dtype=np.float32)
    q_w = np.asarray(q_w, dtype=np.float32)
    k_w = np.asarray(k_w, dtype=np.float32)
    v_w = np.asarray(v_w, dtype=np.float32)
    o_w = np.asarray(o_w, dtype=np.float32)

    # ---- host-side prep (layout/sharding only) ----
    m = np.asarray(mask[:, 0], dtype=bool)                      # [B, T, S]
    act = m.reshape(B, T, ST, 128).any(axis=(1, 3))             # [B, ST]
    act_sets = tuple(tuple(int(j) for j in np.nonzero(act[b])[0]) for b in range(B))

    key = act_sets
    if key not in _PROGRAM_CACHE:
        _PROGRAM_CACHE[key] = _build_program(act_sets)
    nc = _PROGRAM_CACHE[key]

    xT = np.ascontiguousarray(x.reshape(B * T, D).T)            # [D, B*T] f32
    pos = start_positions.astype(np.int64)[:, None] + np.arange(T)[None, :]
    cosT = np.ascontiguousarray(cos[pos].transpose(0, 2, 1))    # [B, HD, T]
    sinT = np.ascontiguousarray(sin[pos].transpose(0, 2, 1))

    mask_tiles = []
    for b in range(B):
        for j in act_sets[b]:
            t1 = (~m[b, :, j * 128:(j + 1) * 128]).T.astype(BF16)
            mask_tiles.append(np.concatenate([t1, t1], axis=1))
    if mask_tiles:
        maskm = np.ascontiguousarray(np.stack(mask_tiles))
    else:
        maskm = np.zeros((1, 128, 2 * T), dtype=BF16)

    in_maps = []
    for c in range(NCORES):
        hsl = slice(c * HPC * HD, (c + 1) * HPC * HD)
        gsl = slice(c * HD, (c + 1) * HD)
        in_maps.append({
            "xT": xT,
            "qw": np.ascontiguousarray(q_w[:, hsl]),
            "kw": np.ascontiguousarray(k_w[:, gsl]),
            "vw": np.ascontiguousarray(v_w[:, gsl]),
            "ow": np.ascontiguousarray(o_w[hsl, :]).astype(BF16),
            "pkT": np.ascontiguousarray(prev_k[:, c].transpose(0, 2, 1)),
            "pv": np.ascontiguousarray(prev_v[:, c]).astype(BF16),
            "cosT": cosT,
            "ones1": np.ones((1, 128), dtype=np.float32),
            "sinT": sinT,
            "maskm": maskm,
        })

    trace = bool(int(os.environ.get("TRNK_TRACE", "0")))
    if trace:
        try:
            import trnprof
            trnprof.install()
        except Exception:
            trace = False
    global LAST_RESULT
    res = run_bass_kernel_spmd(nc, in_maps, list(range(NCORES)), trace=trace)
    LAST_EXEC_TIME_NS = res.exec_time_ns
    LAST_RESULT = res

    # ---- gather / unshard ----
    out = np.zeros((B * T, D), dtype=np.float32)
    for c in range(NCORES):
        out += res.results[c]["out_p"].astype(np.float32)
    out = out.reshape(B, T, D)

    k_all = np.empty((B, G, S, HD), dtype=np.float32)
    v_all = np.empty((B, G, S, HD), dtype=np.float32)
    k_all[:, :, :CACHE] = prev_k
    v_all[:, :, :CACHE] = prev_v
    for c in range(NCORES):
        k_all[:, c, CACHE:] = res.results[c]["knewT"].transpose(0, 2, 1)
        v_all[:, c, CACHE:] = res.results[c]["vnewT"].transpose(0, 2, 1)

    return out, k_all, v_all


# revision 13
# speedup vs baseline: 1.0140x; 1.0140x over previous
"""GQA (ragged_sequence) Trainium2 kernel, 8-core tensor-parallel.

Sharding: heads/KV-groups across the 8 cores (4 query heads + 1 KV group per
core; q_w/o_w split on head axis, k_w/v_w on group axis). Batch stays whole on
every core. o_w is row-split, so each core emits a partial output summed on
host. KV caches shard naturally with the group axis.

Per-core dataflow (all layouts chosen so no on-device transposes are needed
except a tiny PE-transpose for V):
  - qk path in float32r (tf32-ish, ~bf16 speed at N>=512, 16x better accuracy)
  - scores computed transposed: scoresT[s, t] = (k @ qT), softmax sums via
    running bf16 accumulators + a ones-matmul, normalization via a K=1
    broadcast matmul.
  - ctx computed transposed: ctxT[hd, t] = sum_s v[s,hd]^T p[s,t], which feeds
    the o-projection directly (contraction dim on partitions).
"""

import os
import sys
import math

for _p in ("/opt/trn_rl_repo",):
    if _p not in sys.path and os.path.isdir(_p):
        sys.path.insert(0, _p)

import numpy as np
import ml_dtypes
from contextlib import ExitStack

import concourse.bass as bass
import concourse.tile as tile
from concourse import bacc, mybir
from concourse.bass_utils import run_bass_kernel_spmd
from concourse.masks import make_identity

BF16 = ml_dtypes.bfloat16

B, T, D = 4, 512, 4096
H, G, HD = 32, 8, 128
GS = H // G
CACHE = 3584
S = CACHE + T
NCORES = 8
HPC = H // NCORES          # heads per core = 4
KT = D // 128              # 32 contraction tiles
ST = S // 128              # 32 s-tiles
NCT = CACHE // 128         # 28 cache s-tiles
TB = T // 128              # 4 token blocks per batch
OCH = D // 512             # 8 output chunks
SOFTMAX_SCALE = 1.0 / math.sqrt(HD)

_PROGRAM_CACHE = {}
LAST_EXEC_TIME_NS = None
LAST_RESULT = None


def _build_program(act_sets):
    """Build the SPMD Bass program. act_sets: tuple per batch of the s-tile
    indices where the attention mask has any True (mask applied only there)."""
    f32 = mybir.dt.float32
    f32r = mybir.dt.float32r
    bf16 = mybir.dt.bfloat16

    nc = bacc.Bacc("TRN2", target_bir_lowering=False, debug=False,
                   num_devices=NCORES)

    # ---- DRAM I/O (per-core shards) ----
    xT_d = nc.dram_tensor("xT", [D, B * T], f32r, kind="ExternalInput")
    qw_d = nc.dram_tensor("qw", [D, HPC * HD], f32r, kind="ExternalInput")
    kw_d = nc.dram_tensor("kw", [D, HD], f32r, kind="ExternalInput")
    vw_d = nc.dram_tensor("vw", [D, HD], f32r, kind="ExternalInput")
    ow_d = nc.dram_tensor("ow", [HPC * HD, D], bf16, kind="ExternalInput")
    pkT_d = nc.dram_tensor("pkT", [B, HD, CACHE], f32r, kind="ExternalInput")
    pv_d = nc.dram_tensor("pv", [B, CACHE, HD], bf16, kind="ExternalInput")
    cosT_d = nc.dram_tensor("cosT", [B, HD, T], f32, kind="ExternalInput")
    sinT_d = nc.dram_tensor("sinT", [B, HD, T], f32, kind="ExternalInput")
    ones1_d = nc.dram_tensor("ones1", [1, 128], mybir.dt.float32r, kind="ExternalInput")
    n_mask = max(1, sum(len(a) for a in act_sets))
    mask_d = nc.dram_tensor("maskm", [n_mask, 128, 2 * T], bf16, kind="ExternalInput")
    mask_idx = {}
    mi = 0
    for b in range(B):
        for j in act_sets[b]:
            mask_idx[(b, j)] = mi
            mi += 1

    out_d = nc.dram_tensor("out_p", [B * T, D], bf16, kind="ExternalOutput")
    knewT_d = nc.dram_tensor("knewT", [B, HD, T], f32, kind="ExternalOutput")
    vnewT_d = nc.dram_tensor("vnewT", [B, HD, T], f32, kind="ExternalOutput")

    with tile.TileContext(nc) as tc, ExitStack() as ctx:
        P = lambda name, bufs, space="SBUF": ctx.enter_context(
            tc.tile_pool(name=name, bufs=bufs, space=space))

        # NOTE: tile tags default to the tile *name*; each tag gets its own
        # `bufs` slots in the pool. PSUM tags are explicit and budgeted to the
        # 8 banks: qacc(3) + mm(3) + ctxps(1) + small(1).
        p_const = P("const", 1)
        p_kw = P("kw", 1)
        p_vw = P("vw", 1)
        p_ow = P("ow", 1)
        p_xt = P("xt", 4)
        p_qw = P("qws", 4)
        p_tab = P("tab", 2)
        p_mask = P("mask", 2)
        p_kT = P("kT", 1)
        p_vall = P("vall", 1)
        p_qT = P("qT", 2)
        p_p = P("pp", 3)
        p_acc = P("acc", 2)
        p_ctx = P("ctx", 2)
        p_tmp = P("tmp", 2)
        p_stage = P("stage", 2)
        p_bc = P("bc", 2)
        p_recip = P("recip", 1)
        p_out = P("outs", 2)
        ps = P("ps", 1, space="PSUM")

        # ---- constants ----
        ident = p_const.tile([128, 128], bf16)
        make_identity(nc, ident)
        onesb = p_const.tile([128, 1], bf16)
        nc.vector.memset(onesb, 1.0)
        ones1 = p_const.tile([1, 128], f32r)
        nc.scalar.dma_start(ones1, ones1_d.ap())

        # ---- resident weights ----
        kw_t = p_kw.tile([128, KT, HD], f32r)
        nc.gpsimd.dma_start(kw_t, kw_d.ap().rearrange("(kt p) n -> p kt n", p=128))
        vw_t = p_vw.tile([128, KT, HD], f32r)
        nc.gpsimd.dma_start(vw_t, vw_d.ap().rearrange("(kt p) n -> p kt n", p=128))
        ow_tiles = []

        # PSUM layout: tag "big" = 3 slots of [128,1024] f32 (2 banks each) for
        # projection accumulator-pairs / paired score tiles / transients, plus
        # tag "ctx2" = 1 slot [128,1024] holding both heads' ctx accumulators.
        def big(name, shape=(128, 1024), dtype=f32):
            return ps.tile(list(shape), dtype, name=name, tag="big", bufs=3)

        for b in range(B):
            bt = slice(b * T, (b + 1) * T)

            # ======== Phase A: q/k/v projections for batch b (one pass) ========
            # 6 accumulators packed two-per-[128,1024] tile (each half = 1 bank)
            ab_q01 = big("abq01")
            ab_q23 = big("abq23")
            ab_kv = big("abkv")
            halves = [ab_q01[:, 0:512], ab_q01[:, 512:1024],
                      ab_q23[:, 0:512], ab_q23[:, 512:1024],
                      ab_kv[:, 0:512], ab_kv[:, 512:1024]]
            for k in range(KT):
                xt = p_xt.tile([128, T], f32r, name="xt")
                nc.sync.dma_start(xt, xT_d.ap()[k * 128:(k + 1) * 128, bt])
                qwk = p_qw.tile([128, HPC * HD], f32r, name="qwk")
                nc.sync.dma_start(qwk, qw_d.ap()[k * 128:(k + 1) * 128, :])
                first, last = (k == 0), (k == KT - 1)
                for m in range(HPC):
                    nc.tensor.matmul(halves[m], qwk[:, m * 128:(m + 1) * 128], xt,
                                     start=first, stop=last)
                nc.tensor.matmul(halves[4], kw_t[:, k, :], xt, start=first, stop=last)
                nc.tensor.matmul(halves[5], vw_t[:, k, :], xt, start=first, stop=last)

            # ======== Phase A2: RoPE + evacuations ========
            cos_b = p_tab.tile([128, T], f32, name="cosb")
            nc.scalar.dma_start(cos_b, cosT_d.ap()[b])
            sin_b = p_tab.tile([128, T], f32, name="sinb")
            nc.scalar.dma_start(sin_b, sinT_d.ap()[b])
            kT_b = p_kT.tile([128, S], f32r, name="kTb")
            nc.gpsimd.dma_start(kT_b[:, 0:CACHE], pkT_d.ap()[b])
            v_b = p_vall.tile([128, S], bf16, name="vb")
            nc.gpsimd.dma_start(
                v_b[:, 0:CACHE].rearrange("p (j h) -> p j h", h=HD),
                pv_d.ap()[b].rearrange("(j p) h -> p j h", p=128))

            def rope(dst, src):
                # dst = src*cos + rot(src)*sin ; dst[:64] uses -src[64:]*sin
                t1 = p_tmp.tile([128, T], f32, name="t1")
                nc.vector.tensor_mul(t1, src, cos_b)
                t2 = p_tmp.tile([128, T], f32, name="t2")
                nc.vector.tensor_mul(t2[0:64, :], src[64:128, :], sin_b[0:64, :])
                nc.vector.tensor_mul(t2[64:128, :], src[0:64, :], sin_b[64:128, :])
                nc.vector.tensor_sub(dst[0:64, :], t1[0:64, :], t2[0:64, :])
                nc.vector.tensor_add(dst[64:128, :], t1[64:128, :], t2[64:128, :])

            qT_b = []
            for m in range(HPC):
                qt = p_qT.tile([128, T], f32r, name=f"qT{m}")
                rope(qt, halves[m])
                qT_b.append(qt)
            rope(kT_b[:, CACHE:S], halves[4])
            nc.scalar.dma_start(knewT_d.ap()[b],
                                kT_b[:, CACHE:S].bitcast(mybir.dt.float32))
            # v: evacuate vT (f32 out + bf16), transpose to [s, hd]
            vst = p_stage.tile([128, T], f32, name="vst")
            nc.scalar.copy(vst, halves[5])
            nc.scalar.dma_start(vnewT_d.ap()[b], vst)
            vT_sb = p_stage.tile([128, T], bf16, name="vTsb")
            nc.vector.tensor_copy(vT_sb, halves[5])
            for tb in range(TB):
                ps_tr = big("pstr", (128, 128), bf16)
                nc.tensor.transpose(ps_tr, vT_sb[:, tb * 128:(tb + 1) * 128], ident)
                nc.vector.tensor_copy(
                    v_b[:, CACHE + tb * 128:CACHE + (tb + 1) * 128], ps_tr)

            if b == 0:
                # deferred: o_w not needed until Phase C of b=0
                for h in range(HPC):
                    owh = p_ow.tile([128, D], bf16, name=f"ow{h}")
                    nc.gpsimd.dma_start(owh, ow_d.ap()[h * 128:(h + 1) * 128, :])
                    ow_tiles.append(owh)

            # ======== Phase B: attention, heads paired two-per-exp ========
            ctx_sb = []
            for pr in range(HPC // 2):
                h0 = 2 * pr
                ctx2 = ps.tile([128, 1024], f32, name="ctx2", tag="ctx2", bufs=1)
                accs = [p_acc.tile([128, 1024], bf16, name=f"a{i}") for i in range(4)]
                for j in range(ST):
                    sc2 = big("sc2")
                    for hh in range(2):
                        nc.tensor.matmul(sc2[:, hh * 512:(hh + 1) * 512],
                                         kT_b[:, j * 128:(j + 1) * 128],
                                         qT_b[h0 + hh], start=True, stop=True)
                    p2 = p_p.tile([128, 1024], bf16, name="p2")
                    nc.scalar.activation(p2, sc2, mybir.ActivationFunctionType.Exp,
                                         scale=SOFTMAX_SCALE)
                    if (b, j) in mask_idx:
                        mt = p_mask.tile([128, 1024], bf16, name="mt")
                        nc.scalar.dma_start(mt, mask_d.ap()[mask_idx[(b, j)]])
                        nc.vector.tensor_mul(p2, p2, mt)
                    ia = j % 4
                    if j < 4:
                        nc.vector.tensor_copy(accs[ia], p2)
                    else:
                        nc.vector.tensor_add(accs[ia], accs[ia], p2)
                    for hh in range(2):
                        nc.tensor.matmul(ctx2[:, hh * 512:(hh + 1) * 512],
                                         v_b[:, j * 128:(j + 1) * 128],
                                         p2[:, hh * 512:(hh + 1) * 512],
                                         start=(j == 0), stop=(j == ST - 1))
                nc.vector.tensor_add(accs[0], accs[0], accs[1])
                nc.vector.tensor_add(accs[2], accs[2], accs[3])
                nc.vector.tensor_add(accs[0], accs[0], accs[2])
                for hh in range(2):
                    sums_ps = big("sums", (1, T))
                    nc.tensor.matmul(sums_ps, onesb,
                                     accs[0][:, hh * 512:(hh + 1) * 512],
                                     start=True, stop=True)
                    recip32 = p_recip.tile([1, T], f32, name="recip32")
                    rscr = p_recip.tile([1, T], f32, name="rscr")
                    nc.vector.reciprocal_approx_accurate(recip32, sums_ps, rscr)
                    recip = p_recip.tile([1, T], f32r, name="recip")
                    nc.vector.tensor_copy(recip, recip32)
                    bc_ps = big("bcps", (128, T))
                    nc.tensor.matmul(bc_ps, ones1, recip, start=True, stop=True)
                    bc_sb = p_bc.tile([128, T], f32, name="bcsb")
                    nc.vector.tensor_copy(bc_sb, bc_ps)
                    cx = p_ctx.tile([128, T], bf16, name=f"cx{h0 + hh}")
                    nc.vector.tensor_mul(cx, ctx2[:, hh * 512:(hh + 1) * 512], bc_sb)
                    ctx_sb.append(cx)

            # ======== Phase C: partial o-projection for batch b ========
            for tb in range(TB):
                for chk in range(OCH):
                    ops_t = big("ops", (128, 512))
                    for h in range(HPC):
                        nc.tensor.matmul(
                            ops_t, ctx_sb[h][:, tb * 128:(tb + 1) * 128],
                            ow_tiles[h][:, chk * 512:(chk + 1) * 512],
                            start=(h == 0), stop=(h == HPC - 1))
                    o_sb = p_out.tile([128, 512], bf16, name="osb")
                    nc.scalar.copy(o_sb, ops_t)
                    nc.gpsimd.dma_start(
                        out_d.ap()[b * T + tb * 128:b * T + (tb + 1) * 128,
                                   chk * 512:(chk + 1) * 512], o_sb)

    nc.compile()
    return nc


def kernel(x, mask, cos, sin, start_positions, prev_k, prev_v, q_w, k_w, v_w, o_w):
    global LAST_EXEC_TIME_NS
    x = np.asarray(x, dtype=np.float32)
    mask = np.asarray(mask)
    cos = np.asarray(cos, dtype=np.float32)
    sin = np.asarray(sin, dtype=np.float32)
    start_positions = np.asarray(start_positions)
    prev_k = np.asarray(prev_k, dtype=np.float32)
    prev_v = np.asarray(prev_v, dtype=np.float32)
    q_w = np.asarray(q_w, dtype=np.float32)
    k_w = np.asarray(k_w, dtype=np.float32)
    v_w = np.asarray(v_w, dtype=np.float32)
    o_w = np.asarray(o_w, dtype=np.float32)

    # ---- host-side prep (layout/sharding only) ----
    m = np.asarray(mask[:, 0], dtype=bool)                      # [B, T, S]
    act = m.reshape(B, T, ST, 128).any(axis=(1, 3))             # [B, ST]
    act_sets = tuple(tuple(int(j) for j in np.nonzero(act[b])[0]) for b in range(B))

    key = act_sets
    if key not in _PROGRAM_CACHE:
        _PROGRAM_CACHE[key] = _build_program(act_sets)
    nc = _PROGRAM_CACHE[key]

    xT = np.ascontiguousarray(x.reshape(B * T, D).T)            # [D, B*T] f32
    pos = start_positions.astype(np.int64)[:, None] + np.arange(T)[None, :]
    cosT = np.ascontiguousarray(cos[pos].transpose(0, 2, 1))    # [B, HD, T]
    sinT = np.ascontiguousarray(sin[pos].transpose(0, 2, 1))

    mask_tiles = []
    for b in range(B):
        for j in act_sets[b]:
            t1 = (~m[b, :, j * 128:(j + 1) * 128]).T.astype(BF16)
            mask_tiles.append(np.concatenate([t1, t1], axis=1))
    if mask_tiles:
        maskm = np.ascontiguousarray(np.stack(mask_tiles))
    else:
        maskm = np.zeros((1, 128, 2 * T), dtype=BF16)

    in_maps = []
    for c in range(NCORES):
        hsl = slice(c * HPC * HD, (c + 1) * HPC * HD)
        gsl = slice(c * HD, (c + 1) * HD)
        in_maps.append({
            "xT": xT,
            "qw": np.ascontiguousarray(q_w[:, hsl]),
            "kw": np.ascontiguousarray(k_w[:, gsl]),
            "vw": np.ascontiguousarray(v_w[:, gsl]),
            "ow": np.ascontiguousarray(o_w[hsl, :]).astype(BF16),
            "pkT": np.ascontiguousarray(prev_k[:, c].transpose(0, 2, 1)),
            "pv": np.ascontiguousarray(prev_v[:, c]).astype(BF16),
            "cosT": cosT,
            "ones1": np.ones((1, 128), dtype=np.float32),
            "sinT": sinT,
            "maskm": maskm,
        })

    trace = bool(int(os.environ.get("TRNK_TRACE", "0")))
    if trace:
        try:
            import trnprof
            trnprof.install()
        except Exception:
            trace = False
    global LAST_RESULT
    res = run_bass_kernel_spmd(nc, in_maps, list(range(NCORES)), trace=trace)
    LAST_EXEC_TIME_NS = res.exec_time_ns
    LAST_RESULT = res

    # ---- gather / unshard ----
    out = np.zeros((B * T, D), dtype=np.float32)
    for c in range(NCORES):
        out += res.results[c]["out_p"].astype(np.float32)
    out = out.reshape(B, T, D)

    k_all = np.empty((B, G, S, HD), dtype=np.float32)
    v_all = np.empty((B, G, S, HD), dtype=np.float32)
    k_all[:, :, :CACHE] = prev_k
    v_all[:, :, :CACHE] = prev_v
    for c in range(NCORES):
        k_all[:, c, CACHE:] = res.results[c]["knewT"].transpose(0, 2, 1)
        v_all[:, c, CACHE:] = res.results[c]["vnewT"].transpose(0, 2, 1)

    return out, k_all, v_all


# revision 14
# speedup vs baseline: 1.0222x; 1.0080x over previous
"""GQA (ragged_sequence) Trainium2 kernel, 8-core tensor-parallel.

Sharding: heads/KV-groups across the 8 cores (4 query heads + 1 KV group per
core; q_w/o_w split on head axis, k_w/v_w on group axis). Batch stays whole on
every core. o_w is row-split, so each core emits a partial output summed on
host. KV caches shard naturally with the group axis.

Per-core dataflow (all layouts chosen so no on-device transposes are needed
except a tiny PE-transpose for V):
  - qk path in float32r (tf32-ish, ~bf16 speed at N>=512, 16x better accuracy)
  - scores computed transposed: scoresT[s, t] = (k @ qT), softmax sums via
    running bf16 accumulators + a ones-matmul, normalization via a K=1
    broadcast matmul.
  - ctx computed transposed: ctxT[hd, t] = sum_s v[s,hd]^T p[s,t], which feeds
    the o-projection directly (contraction dim on partitions).
"""

import os
import sys
import math

for _p in ("/opt/trn_rl_repo",):
    if _p not in sys.path and os.path.isdir(_p):
        sys.path.insert(0, _p)

import numpy as np
import ml_dtypes
from contextlib import ExitStack

import concourse.bass as bass
import concourse.tile as tile
from concourse import bacc, mybir
from concourse.bass_utils import run_bass_kernel_spmd
from concourse.masks import make_identity

BF16 = ml_dtypes.bfloat16

B, T, D = 4, 512, 4096
H, G, HD = 32, 8, 128
GS = H // G
CACHE = 3584
S = CACHE + T
NCORES = 8
HPC = H // NCORES          # heads per core = 4
KT = D // 128              # 32 contraction tiles
ST = S // 128              # 32 s-tiles
NCT = CACHE // 128         # 28 cache s-tiles
TB = T // 128              # 4 token blocks per batch
OCH = D // 512             # 8 output chunks
SOFTMAX_SCALE = 1.0 / math.sqrt(HD)

_PROGRAM_CACHE = {}
LAST_EXEC_TIME_NS = None
LAST_RESULT = None


def _build_program(act_sets):
    """Build the SPMD Bass program. act_sets: tuple per batch of the s-tile
    indices where the attention mask has any True (mask applied only there)."""
    f32 = mybir.dt.float32
    f32r = mybir.dt.float32r
    bf16 = mybir.dt.bfloat16

    nc = bacc.Bacc("TRN2", target_bir_lowering=False, debug=False,
                   num_devices=NCORES)

    # ---- DRAM I/O (per-core shards) ----
    xT_d = nc.dram_tensor("xT", [D, B * T], f32r, kind="ExternalInput")
    qw_d = nc.dram_tensor("qw", [D, HPC * HD], f32r, kind="ExternalInput")
    kw_d = nc.dram_tensor("kw", [D, HD], f32r, kind="ExternalInput")
    vw_d = nc.dram_tensor("vw", [D, HD], f32r, kind="ExternalInput")
    ow_d = nc.dram_tensor("ow", [HPC * HD, D], bf16, kind="ExternalInput")
    pkT_d = nc.dram_tensor("pkT", [B, HD, CACHE], f32r, kind="ExternalInput")
    pv_d = nc.dram_tensor("pv", [B, CACHE, HD], bf16, kind="ExternalInput")
    cosT_d = nc.dram_tensor("cosT", [B, HD, T], f32, kind="ExternalInput")
    sinT_d = nc.dram_tensor("sinT", [B, HD, T], f32, kind="ExternalInput")
    ones1_d = nc.dram_tensor("ones1", [1, 128], mybir.dt.float32r, kind="ExternalInput")
    n_mask = max(1, sum(len(a) for a in act_sets))
    mask_d = nc.dram_tensor("maskm", [n_mask, 128, 2 * T], bf16, kind="ExternalInput")
    mask_idx = {}
    mi = 0
    for b in range(B):
        for j in act_sets[b]:
            mask_idx[(b, j)] = mi
            mi += 1

    out_d = nc.dram_tensor("out_p", [B * T, D], bf16, kind="ExternalOutput")
    knewT_d = nc.dram_tensor("knewT", [B, HD, T], f32, kind="ExternalOutput")
    vnewT_d = nc.dram_tensor("vnewT", [B, HD, T], f32, kind="ExternalOutput")

    with tile.TileContext(nc) as tc, ExitStack() as ctx:
        P = lambda name, bufs, space="SBUF": ctx.enter_context(
            tc.tile_pool(name=name, bufs=bufs, space=space))

        # NOTE: tile tags default to the tile *name*; each tag gets its own
        # `bufs` slots in the pool. PSUM tags are explicit and budgeted to the
        # 8 banks: qacc(3) + mm(3) + ctxps(1) + small(1).
        p_const = P("const", 1)
        p_kw = P("kw", 1)
        p_vw = P("vw", 1)
        p_ow = P("ow", 1)
        p_xt = P("xt", 4)
        p_qw = P("qws", 4)
        p_tab = P("tab", 2)
        p_mask = P("mask", 2)
        p_kT = P("kT", 1)
        p_vall = P("vall", 1)
        p_qT = P("qT", 2)
        p_p = P("pp", 3)
        p_acc = P("acc", 2)
        p_ctx = P("ctx", 2)
        p_tmp = P("tmp", 2)
        p_stage = P("stage", 2)
        p_bc = P("bc", 2)
        p_recip = P("recip", 1)
        p_out = P("outs", 2)
        ps = P("ps", 1, space="PSUM")

        # ---- constants ----
        ident = p_const.tile([128, 128], bf16)
        make_identity(nc, ident)
        onesb = p_const.tile([128, 1], bf16)
        nc.vector.memset(onesb, 1.0)
        ones1 = p_const.tile([1, 128], f32r)
        nc.scalar.dma_start(ones1, ones1_d.ap())

        # ---- resident weights ----
        kw_t = p_kw.tile([128, KT, HD], f32r)
        nc.gpsimd.dma_start(kw_t, kw_d.ap().rearrange("(kt p) n -> p kt n", p=128))
        vw_t = p_vw.tile([128, KT, HD], f32r)
        nc.gpsimd.dma_start(vw_t, vw_d.ap().rearrange("(kt p) n -> p kt n", p=128))
        ow_tiles = []

        # PSUM tags: qacc(3 banks, proj accumulators) + mm(3, scores/o-proj)
        # + ctxps(1) + small(1) = 8 banks. Separate tags let phase A(b+1)
        # overlap attention(b) instead of serializing through shared slots.
        for b in range(B):
            bt = slice(b * T, (b + 1) * T)

            # ======== Phase A: two passes of 3 accumulators ========
            cos_b = p_tab.tile([128, T], f32, name="cosb")
            nc.scalar.dma_start(cos_b, cosT_d.ap()[b])
            sin_b = p_tab.tile([128, T], f32, name="sinb")
            nc.scalar.dma_start(sin_b, sinT_d.ap()[b])
            kT_b = p_kT.tile([128, S], f32r, name="kTb")
            nc.gpsimd.dma_start(kT_b[:, 0:CACHE], pkT_d.ap()[b])
            v_b = p_vall.tile([128, S], bf16, name="vb")
            nc.gpsimd.dma_start(
                v_b[:, 0:CACHE].rearrange("p (j h) -> p j h", h=HD),
                pv_d.ap()[b].rearrange("(j p) h -> p j h", p=128))

            def rope(dst, src_ap):
                t1 = p_tmp.tile([128, T], f32, name="t1")
                nc.vector.tensor_mul(t1, src_ap, cos_b)
                t2 = p_tmp.tile([128, T], f32, name="t2")
                nc.vector.tensor_mul(t2[0:64, :], src_ap[64:128, :], sin_b[0:64, :])
                nc.vector.tensor_mul(t2[64:128, :], src_ap[0:64, :], sin_b[64:128, :])
                nc.vector.tensor_sub(dst[0:64, :], t1[0:64, :], t2[0:64, :])
                nc.vector.tensor_add(dst[64:128, :], t1[64:128, :], t2[64:128, :])

            qT_b = []
            for hp in range(2):
                ps_a = ps.tile([128, T], f32, name="psa", tag="qacc", bufs=3)
                ps_b2 = ps.tile([128, T], f32, name="psb", tag="qacc", bufs=3)
                ps_c = ps.tile([128, T], f32, name="psc", tag="qacc", bufs=3)
                qcols = slice(hp * 256, hp * 256 + 256)
                kvw = kw_t if hp == 0 else vw_t
                for k in range(KT):
                    xt = p_xt.tile([128, T], f32r, name="xt")
                    nc.sync.dma_start(xt, xT_d.ap()[k * 128:(k + 1) * 128, bt])
                    qwk = p_qw.tile([128, 256], f32r, name="qwk")
                    nc.sync.dma_start(qwk, qw_d.ap()[k * 128:(k + 1) * 128, qcols])
                    first, last = (k == 0), (k == KT - 1)
                    nc.tensor.matmul(ps_a, qwk[:, 0:128], xt, start=first, stop=last)
                    nc.tensor.matmul(ps_b2, qwk[:, 128:256], xt, start=first, stop=last)
                    nc.tensor.matmul(ps_c, kvw[:, k, :], xt, start=first, stop=last)

                for m, srcp in ((2 * hp, ps_a), (2 * hp + 1, ps_b2)):
                    qt = p_qT.tile([128, T], f32r, name=f"qT{m}")
                    rope(qt, srcp)
                    qT_b.append(qt)

                if hp == 0:
                    rope(kT_b[:, CACHE:S], ps_c)
                    nc.scalar.dma_start(knewT_d.ap()[b],
                                        kT_b[:, CACHE:S].bitcast(mybir.dt.float32))
                else:
                    vst = p_stage.tile([128, T], f32, name="vst")
                    nc.scalar.copy(vst, ps_c)
                    nc.scalar.dma_start(vnewT_d.ap()[b], vst)
                    vT_sb = p_stage.tile([128, T], bf16, name="vTsb")
                    nc.vector.tensor_copy(vT_sb, ps_c)
                    for tb in range(TB):
                        ps_tr = ps.tile([128, 128], bf16, name="pstr",
                                        tag="small", bufs=1)
                        nc.tensor.transpose(ps_tr, vT_sb[:, tb * 128:(tb + 1) * 128],
                                            ident)
                        nc.vector.tensor_copy(
                            v_b[:, CACHE + tb * 128:CACHE + (tb + 1) * 128], ps_tr)

            if b == 0:
                for h in range(HPC):
                    owh = p_ow.tile([128, D], bf16, name=f"ow{h}")
                    nc.gpsimd.dma_start(owh, ow_d.ap()[h * 128:(h + 1) * 128, :])
                    ow_tiles.append(owh)

            # ======== Phase B: attention per local head ========
            ctx_sb = []
            for h in range(HPC):
                accs = [p_acc.tile([128, T], bf16, name=f"a{i}") for i in range(4)]
                ctx_ps = ps.tile([128, T], f32, name="ctxps", tag="ctxps", bufs=1)
                for j in range(ST):
                    sc = ps.tile([128, T], f32, name="sc", tag="mm", bufs=3)
                    nc.tensor.matmul(sc, kT_b[:, j * 128:(j + 1) * 128], qT_b[h],
                                     start=True, stop=True)
                    p_j = p_p.tile([128, T], bf16, name="pj")
                    nc.scalar.activation(p_j, sc, mybir.ActivationFunctionType.Exp,
                                         scale=SOFTMAX_SCALE)
                    if (b, j) in mask_idx:
                        mt = p_mask.tile([128, T], bf16, name="mt")
                        nc.scalar.dma_start(mt, mask_d.ap()[mask_idx[(b, j)], :, 0:T])
                        nc.vector.tensor_mul(p_j, p_j, mt)
                    ia = j % 4
                    if j < 4:
                        nc.vector.tensor_copy(accs[ia], p_j)
                    else:
                        nc.vector.tensor_add(accs[ia], accs[ia], p_j)
                    nc.tensor.matmul(ctx_ps, v_b[:, j * 128:(j + 1) * 128], p_j,
                                     start=(j == 0), stop=(j == ST - 1))
                nc.vector.tensor_add(accs[0], accs[0], accs[1])
                nc.vector.tensor_add(accs[2], accs[2], accs[3])
                nc.vector.tensor_add(accs[0], accs[0], accs[2])
                sums_ps = ps.tile([1, T], f32, name="sums", tag="small", bufs=1)
                nc.tensor.matmul(sums_ps, onesb, accs[0], start=True, stop=True)
                recip32 = p_recip.tile([1, T], f32, name="recip32")
                rscr = p_recip.tile([1, T], f32, name="rscr")
                nc.vector.reciprocal_approx_accurate(recip32, sums_ps, rscr)
                recip = p_recip.tile([1, T], f32r, name="recip")
                nc.vector.tensor_copy(recip, recip32)
                bc_ps = ps.tile([128, T], f32, name="bcps", tag="small", bufs=1)
                nc.tensor.matmul(bc_ps, ones1, recip, start=True, stop=True)
                bc_sb = p_bc.tile([128, T], f32, name="bcsb")
                nc.vector.tensor_copy(bc_sb, bc_ps)
                cx = p_ctx.tile([128, T], bf16, name=f"cx{h}")
                nc.vector.tensor_mul(cx, ctx_ps, bc_sb)
                ctx_sb.append(cx)

            # ======== Phase C: partial o-projection ========
            for tb in range(TB):
                for chk in range(OCH):
                    ops_t = ps.tile([128, 512], f32, name="ops", tag="mm", bufs=3)
                    for h in range(HPC):
                        nc.tensor.matmul(
                            ops_t, ctx_sb[h][:, tb * 128:(tb + 1) * 128],
                            ow_tiles[h][:, chk * 512:(chk + 1) * 512],
                            start=(h == 0), stop=(h == HPC - 1))
                    o_sb = p_out.tile([128, 512], bf16, name="osb")
                    nc.scalar.copy(o_sb, ops_t)
                    nc.gpsimd.dma_start(
                        out_d.ap()[b * T + tb * 128:b * T + (tb + 1) * 128,
                                   chk * 512:(chk + 1) * 512], o_sb)

    nc.compile()
    return nc


def kernel(x, mask, cos, sin, start_positions, prev_k, prev_v, q_w, k_w, v_w, o_w):
    global LAST_EXEC_TIME_NS
    x = np.asarray(x, dtype=np.float32)
    mask = np.asarray(mask)
    cos = np.asarray(cos, dtype=np.float32)
    sin = np.asarray(sin, dtype=np.float32)
    start_positions = np.asarray(start_positions)
    prev_k = np.asarray(prev_k, dtype=np.float32)
    prev_v = np.asarray(prev_v, dtype=np.float32)
    q_w = np.asarray(q_w, dtype=np.float32)
    k_w = np.asarray(k_w, dtype=np.float32)
    v_w = np.asarray(v_w, dtype=np.float32)
    o_w = np.asarray(o_w, dtype=np.float32)

    # ---- host-side prep (layout/sharding only) ----
    m = np.asarray(mask[:, 0], dtype=bool)                      # [B, T, S]
    act = m.reshape(B, T, ST, 128).any(axis=(1, 3))             # [B, ST]
    act_sets = tuple(tuple(int(j) for j in np.nonzero(act[b])[0]) for b in range(B))

    key = act_sets
    if key not in _PROGRAM_CACHE:
        _PROGRAM_CACHE[key] = _build_program(act_sets)
    nc = _PROGRAM_CACHE[key]

    xT = np.ascontiguousarray(x.reshape(B * T, D).T)            # [D, B*T] f32
    pos = start_positions.astype(np.int64)[:, None] + np.arange(T)[None, :]
    cosT = np.ascontiguousarray(cos[pos].transpose(0, 2, 1))    # [B, HD, T]
    sinT = np.ascontiguousarray(sin[pos].transpose(0, 2, 1))

    mask_tiles = []
    for b in range(B):
        for j in act_sets[b]:
            t1 = (~m[b, :, j * 128:(j + 1) * 128]).T.astype(BF16)
            mask_tiles.append(np.concatenate([t1, t1], axis=1))
    if mask_tiles:
        maskm = np.ascontiguousarray(np.stack(mask_tiles))
    else:
        maskm = np.zeros((1, 128, 2 * T), dtype=BF16)

    in_maps = []
    for c in range(NCORES):
        hsl = slice(c * HPC * HD, (c + 1) * HPC * HD)
        gsl = slice(c * HD, (c + 1) * HD)
        in_maps.append({
            "xT": xT,
            "qw": np.ascontiguousarray(q_w[:, hsl]),
            "kw": np.ascontiguousarray(k_w[:, gsl]),
            "vw": np.ascontiguousarray(v_w[:, gsl]),
            "ow": np.ascontiguousarray(o_w[hsl, :]).astype(BF16),
            "pkT": np.ascontiguousarray(prev_k[:, c].transpose(0, 2, 1)),
            "pv": np.ascontiguousarray(prev_v[:, c]).astype(BF16),
            "cosT": cosT,
            "ones1": np.ones((1, 128), dtype=np.float32),
            "sinT": sinT,
            "maskm": maskm,
        })

    trace = bool(int(os.environ.get("TRNK_TRACE", "0")))
    if trace:
        try:
            import trnprof
            trnprof.install()
        except Exception:
            trace = False
    global LAST_RESULT
    res = run_bass_kernel_spmd(nc, in_maps, list(range(NCORES)), trace=trace)
    LAST_EXEC_TIME_NS = res.exec_time_ns
    LAST_RESULT = res

    # ---- gather / unshard ----
    out = np.zeros((B * T, D), dtype=np.float32)
    for c in range(NCORES):
        out += res.results[c]["out_p"].astype(np.float32)
    out = out.reshape(B, T, D)

    k_all = np.empty((B, G, S, HD), dtype=np.float32)
    v_all = np.empty((B, G, S, HD), dtype=np.float32)
    k_all[:, :, :CACHE] = prev_k
    v_all[:, :, :CACHE] = prev_v
    for c in range(NCORES):
        k_all[:, c, CACHE:] = res.results[c]["knewT"].transpose(0, 2, 1)
        v_all[:, c, CACHE:] = res.results[c]["vnewT"].transpose(0, 2, 1)

    return out, k_all, v_all


# revision 15
# speedup vs baseline: 1.0602x; 1.0372x over previous
"""GQA (ragged_sequence) Trainium2 kernel, 8-core tensor-parallel.

Sharding: heads/KV-groups across the 8 cores (4 query heads + 1 KV group per
core; q_w/o_w split on head axis, k_w/v_w on group axis). Batch stays whole on
every core. o_w is row-split, so each core emits a partial output summed on
host. KV caches shard naturally with the group axis.

Per-core dataflow (all layouts chosen so no on-device transposes are needed
except a tiny PE-transpose for V):
  - qk path in float32r (tf32-ish, ~bf16 speed at N>=512, 16x better accuracy)
  - scores computed transposed: scoresT[s, t] = (k @ qT), softmax sums via
    running bf16 accumulators + a ones-matmul, normalization via a K=1
    broadcast matmul.
  - ctx computed transposed: ctxT[hd, t] = sum_s v[s,hd]^T p[s,t], which feeds
    the o-projection directly (contraction dim on partitions).
"""

import os
import sys
import math

for _p in ("/opt/trn_rl_repo",):
    if _p not in sys.path and os.path.isdir(_p):
        sys.path.insert(0, _p)

import numpy as np
import ml_dtypes
from contextlib import ExitStack

import concourse.bass as bass
import concourse.tile as tile
from concourse import bacc, mybir
from concourse.bass_utils import run_bass_kernel_spmd
from concourse.masks import make_identity

BF16 = ml_dtypes.bfloat16

B, T, D = 4, 512, 4096
H, G, HD = 32, 8, 128
GS = H // G
CACHE = 3584
S = CACHE + T
NCORES = 8
HPC = H // NCORES          # heads per core = 4
KT = D // 128              # 32 contraction tiles
ST = S // 128              # 32 s-tiles
NCT = CACHE // 128         # 28 cache s-tiles
TB = T // 128              # 4 token blocks per batch
OCH = D // 512             # 8 output chunks
SOFTMAX_SCALE = 1.0 / math.sqrt(HD)

_PROGRAM_CACHE = {}
LAST_EXEC_TIME_NS = None
LAST_RESULT = None


def _build_program(act_sets):
    """Build the SPMD Bass program. act_sets: tuple per batch of the s-tile
    indices where the attention mask has any True (mask applied only there)."""
    f32 = mybir.dt.float32
    f32r = mybir.dt.float32r
    bf16 = mybir.dt.bfloat16

    nc = bacc.Bacc("TRN2", target_bir_lowering=False, debug=False,
                   num_devices=NCORES)

    # ---- DRAM I/O (per-core shards) ----
    xT_d = nc.dram_tensor("xT", [D, B * T], f32r, kind="ExternalInput")
    qw_d = nc.dram_tensor("qw", [D, HPC * HD], f32r, kind="ExternalInput")
    kw_d = nc.dram_tensor("kw", [D, HD], f32r, kind="ExternalInput")
    vw_d = nc.dram_tensor("vw", [D, HD], f32r, kind="ExternalInput")
    ow_d = nc.dram_tensor("ow", [HPC * HD, D], bf16, kind="ExternalInput")
    pkT_d = nc.dram_tensor("pkT", [B, HD, CACHE], f32r, kind="ExternalInput")
    pv_d = nc.dram_tensor("pv", [B, CACHE, HD], bf16, kind="ExternalInput")
    cosT_d = nc.dram_tensor("cosT", [B, HD, T], f32, kind="ExternalInput")
    sinT_d = nc.dram_tensor("sinT", [B, HD, T], f32, kind="ExternalInput")
    ones1_d = nc.dram_tensor("ones1", [1, 128], mybir.dt.float32r, kind="ExternalInput")
    n_mask = max(1, sum(len(a) for a in act_sets))
    mask_d = nc.dram_tensor("maskm", [n_mask, 128, 2 * T], bf16, kind="ExternalInput")
    mask_idx = {}
    mi = 0
    for b in range(B):
        for j in act_sets[b]:
            mask_idx[(b, j)] = mi
            mi += 1

    out_d = nc.dram_tensor("out_p", [B * T, D], bf16, kind="ExternalOutput")
    knewT_d = nc.dram_tensor("knewT", [B, HD, T], f32, kind="ExternalOutput")
    vnewT_d = nc.dram_tensor("vnewT", [B, HD, T], f32, kind="ExternalOutput")

    with tile.TileContext(nc) as tc, ExitStack() as ctx:
        P = lambda name, bufs, space="SBUF": ctx.enter_context(
            tc.tile_pool(name=name, bufs=bufs, space=space))

        # NOTE: tile tags default to the tile *name*; each tag gets its own
        # `bufs` slots in the pool. PSUM tags are explicit and budgeted to the
        # 8 banks: qacc(3) + mm(3) + ctxps(1) + small(1).
        p_const = P("const", 1)
        p_kw = P("kw", 1)
        p_vw = P("vw", 1)
        p_ow = P("ow", 1)
        p_xt = P("xt", 4)
        p_qw = P("qws", 4)
        p_tab = P("tab", 2)
        p_mask = P("mask", 2)
        p_kT = P("kT", 1)
        p_vall = P("vall", 1)
        p_qT = P("qT", 2)
        p_p = P("pp", 3)
        p_acc = P("acc", 2)
        p_ctx = P("ctx", 2)
        p_tmp = P("tmp", 2)
        p_stage = P("stage", 2)
        p_bc = P("bc", 2)
        p_recip = P("recip", 1)
        p_out = P("outs", 2)
        ps = P("ps", 1, space="PSUM")

        # ---- constants ----
        ident = p_const.tile([128, 128], bf16)
        make_identity(nc, ident)
        onesb = p_const.tile([128, 1], bf16)
        nc.vector.memset(onesb, 1.0)
        ones1 = p_const.tile([1, 128], f32r)
        nc.sync.dma_start(ones1, ones1_d.ap())

        # ---- resident weights ----
        kw_t = p_kw.tile([128, KT, HD], f32r)
        nc.sync.dma_start(kw_t, kw_d.ap().rearrange("(kt p) n -> p kt n", p=128))
        vw_t = p_vw.tile([128, KT, HD], f32r)
        nc.sync.dma_start(vw_t, vw_d.ap().rearrange("(kt p) n -> p kt n", p=128))
        ow_tiles = []

        # PSUM tags: qacc(3 banks, proj accumulators) + mm(3, scores/o-proj)
        # + ctxps(1) + small(1) = 8 banks. Separate tags let phase A(b+1)
        # overlap attention(b) instead of serializing through shared slots.
        for b in range(B):
            bt = slice(b * T, (b + 1) * T)

            # ======== Phase A: two passes of 3 accumulators ========
            cos_b = p_tab.tile([128, T], f32, name="cosb")
            nc.sync.dma_start(cos_b, cosT_d.ap()[b])
            sin_b = p_tab.tile([128, T], f32, name="sinb")
            nc.sync.dma_start(sin_b, sinT_d.ap()[b])
            kT_b = p_kT.tile([128, S], f32r, name="kTb")
            nc.sync.dma_start(kT_b[:, 0:CACHE], pkT_d.ap()[b])
            v_b = p_vall.tile([128, S], bf16, name="vb")
            nc.sync.dma_start(
                v_b[:, 0:CACHE].rearrange("p (j h) -> p j h", h=HD),
                pv_d.ap()[b].rearrange("(j p) h -> p j h", p=128))

            def rope(dst, src_ap):
                t1 = p_tmp.tile([128, T], f32, name="t1")
                nc.vector.tensor_mul(t1, src_ap, cos_b)
                t2 = p_tmp.tile([128, T], f32, name="t2")
                nc.vector.tensor_mul(t2[0:64, :], src_ap[64:128, :], sin_b[0:64, :])
                nc.vector.tensor_mul(t2[64:128, :], src_ap[0:64, :], sin_b[64:128, :])
                nc.vector.tensor_sub(dst[0:64, :], t1[0:64, :], t2[0:64, :])
                nc.vector.tensor_add(dst[64:128, :], t1[64:128, :], t2[64:128, :])

            qT_b = []
            for hp in range(2):
                ps_a = ps.tile([128, T], f32, name="psa", tag="qacc", bufs=3)
                ps_b2 = ps.tile([128, T], f32, name="psb", tag="qacc", bufs=3)
                ps_c = ps.tile([128, T], f32, name="psc", tag="qacc", bufs=3)
                qcols = slice(hp * 256, hp * 256 + 256)
                kvw = kw_t if hp == 0 else vw_t
                for k in range(KT):
                    xt = p_xt.tile([128, T], f32r, name="xt")
                    nc.sync.dma_start(xt, xT_d.ap()[k * 128:(k + 1) * 128, bt])
                    qwk = p_qw.tile([128, 256], f32r, name="qwk")
                    nc.sync.dma_start(qwk, qw_d.ap()[k * 128:(k + 1) * 128, qcols])
                    first, last = (k == 0), (k == KT - 1)
                    nc.tensor.matmul(ps_a, qwk[:, 0:128], xt, start=first, stop=last)
                    nc.tensor.matmul(ps_b2, qwk[:, 128:256], xt, start=first, stop=last)
                    nc.tensor.matmul(ps_c, kvw[:, k, :], xt, start=first, stop=last)

                for m, srcp in ((2 * hp, ps_a), (2 * hp + 1, ps_b2)):
                    qt = p_qT.tile([128, T], f32r, name=f"qT{m}")
                    rope(qt, srcp)
                    qT_b.append(qt)

                if hp == 0:
                    rope(kT_b[:, CACHE:S], ps_c)
                    nc.sync.dma_start(knewT_d.ap()[b],
                                        kT_b[:, CACHE:S].bitcast(mybir.dt.float32))
                else:
                    vst = p_stage.tile([128, T], f32, name="vst")
                    nc.scalar.copy(vst, ps_c)
                    nc.sync.dma_start(vnewT_d.ap()[b], vst)
                    vT_sb = p_stage.tile([128, T], bf16, name="vTsb")
                    nc.vector.tensor_copy(vT_sb, ps_c)
                    for tb in range(TB):
                        ps_tr = ps.tile([128, 128], bf16, name="pstr",
                                        tag="small", bufs=1)
                        nc.tensor.transpose(ps_tr, vT_sb[:, tb * 128:(tb + 1) * 128],
                                            ident)
                        nc.vector.tensor_copy(
                            v_b[:, CACHE + tb * 128:CACHE + (tb + 1) * 128], ps_tr)

            if b == 0:
                for h in range(HPC):
                    owh = p_ow.tile([128, D], bf16, name=f"ow{h}")
                    nc.sync.dma_start(owh, ow_d.ap()[h * 128:(h + 1) * 128, :])
                    ow_tiles.append(owh)

            # ======== Phase B: attention per local head ========
            ctx_sb = []
            for h in range(HPC):
                accs = [p_acc.tile([128, T], bf16, name=f"a{i}") for i in range(4)]
                ctx_ps = ps.tile([128, T], f32, name="ctxps", tag="ctxps", bufs=1)
                for j in range(ST):
                    sc = ps.tile([128, T], f32, name="sc", tag="mm", bufs=3)
                    nc.tensor.matmul(sc, kT_b[:, j * 128:(j + 1) * 128], qT_b[h],
                                     start=True, stop=True)
                    p_j = p_p.tile([128, T], bf16, name="pj")
                    nc.scalar.activation(p_j, sc, mybir.ActivationFunctionType.Exp,
                                         scale=SOFTMAX_SCALE)
                    if (b, j) in mask_idx:
                        mt = p_mask.tile([128, T], bf16, name="mt")
                        nc.sync.dma_start(mt, mask_d.ap()[mask_idx[(b, j)], :, 0:T])
                        nc.vector.tensor_mul(p_j, p_j, mt)
                    ia = j % 4
                    if j < 4:
                        nc.vector.tensor_copy(accs[ia], p_j)
                    else:
                        nc.vector.tensor_add(accs[ia], accs[ia], p_j)
                    nc.tensor.matmul(ctx_ps, v_b[:, j * 128:(j + 1) * 128], p_j,
                                     start=(j == 0), stop=(j == ST - 1))
                nc.vector.tensor_add(accs[0], accs[0], accs[1])
                nc.vector.tensor_add(accs[2], accs[2], accs[3])
                nc.vector.tensor_add(accs[0], accs[0], accs[2])
                sums_ps = ps.tile([1, T], f32, name="sums", tag="small", bufs=1)
                nc.tensor.matmul(sums_ps, onesb, accs[0], start=True, stop=True)
                recip32 = p_recip.tile([1, T], f32, name="recip32")
                rscr = p_recip.tile([1, T], f32, name="rscr")
                nc.vector.reciprocal_approx_accurate(recip32, sums_ps, rscr)
                recip = p_recip.tile([1, T], f32r, name="recip")
                nc.vector.tensor_copy(recip, recip32)
                bc_ps = ps.tile([128, T], f32, name="bcps", tag="small", bufs=1)
                nc.tensor.matmul(bc_ps, ones1, recip, start=True, stop=True)
                bc_sb = p_bc.tile([128, T], f32, name="bcsb")
                nc.vector.tensor_copy(bc_sb, bc_ps)
                cx = p_ctx.tile([128, T], bf16, name=f"cx{h}")
                nc.vector.tensor_mul(cx, ctx_ps, bc_sb)
                ctx_sb.append(cx)

            # ======== Phase C: partial o-projection ========
            for tb in range(TB):
                for chk in range(OCH):
                    ops_t = ps.tile([128, 512], f32, name="ops", tag="mm", bufs=3)
                    for h in range(HPC):
                        nc.tensor.matmul(
                            ops_t, ctx_sb[h][:, tb * 128:(tb + 1) * 128],
                            ow_tiles[h][:, chk * 512:(chk + 1) * 512],
                            start=(h == 0), stop=(h == HPC - 1))
                    o_sb = p_out.tile([128, 512], bf16, name="osb")
                    nc.vector.tensor_copy(o_sb, ops_t)
                    nc.sync.dma_start(
                        out_d.ap()[b * T + tb * 128:b * T + (tb + 1) * 128,
                                   chk * 512:(chk + 1) * 512], o_sb)

    nc.compile()
    return nc


def kernel(x, mask, cos, sin, start_positions, prev_k, prev_v, q_w, k_w, v_w, o_w):
    global LAST_EXEC_TIME_NS
    x = np.asarray(x, dtype=np.float32)
    mask = np.asarray(mask)
    cos = np.asarray(cos, dtype=np.float32)
    sin = np.asarray(sin, dtype=np.float32)
    start_positions = np.asarray(start_positions)
    prev_k = np.asarray(prev_k, dtype=np.float32)
    prev_v = np.asarray(prev_v, dtype=np.float32)
    q_w = np.asarray(q_w, dtype=np.float32)
    k_w = np.asarray(k_w, dtype=np.float32)
    v_w = np.asarray(v_w, dtype=np.float32)
    o_w = np.asarray(o_w, dtype=np.float32)

    # ---- host-side prep (layout/sharding only) ----
    m = np.asarray(mask[:, 0], dtype=bool)                      # [B, T, S]
    act = m.reshape(B, T, ST, 128).any(axis=(1, 3))             # [B, ST]
    act_sets = tuple(tuple(int(j) for j in np.nonzero(act[b])[0]) for b in range(B))

    key = act_sets
    if key not in _PROGRAM_CACHE:
        _PROGRAM_CACHE[key] = _build_program(act_sets)
    nc = _PROGRAM_CACHE[key]

    xT = np.ascontiguousarray(x.reshape(B * T, D).T)            # [D, B*T] f32
    pos = start_positions.astype(np.int64)[:, None] + np.arange(T)[None, :]
    cosT = np.ascontiguousarray(cos[pos].transpose(0, 2, 1))    # [B, HD, T]
    sinT = np.ascontiguousarray(sin[pos].transpose(0, 2, 1))

    mask_tiles = []
    for b in range(B):
        for j in act_sets[b]:
            t1 = (~m[b, :, j * 128:(j + 1) * 128]).T.astype(BF16)
            mask_tiles.append(np.concatenate([t1, t1], axis=1))
    if mask_tiles:
        maskm = np.ascontiguousarray(np.stack(mask_tiles))
    else:
        maskm = np.zeros((1, 128, 2 * T), dtype=BF16)

    in_maps = []
    for c in range(NCORES):
        hsl = slice(c * HPC * HD, (c + 1) * HPC * HD)
        gsl = slice(c * HD, (c + 1) * HD)
        in_maps.append({
            "xT": xT,
            "qw": np.ascontiguousarray(q_w[:, hsl]),
            "kw": np.ascontiguousarray(k_w[:, gsl]),
            "vw": np.ascontiguousarray(v_w[:, gsl]),
            "ow": np.ascontiguousarray(o_w[hsl, :]).astype(BF16),
            "pkT": np.ascontiguousarray(prev_k[:, c].transpose(0, 2, 1)),
            "pv": np.ascontiguousarray(prev_v[:, c]).astype(BF16),
            "cosT": cosT,
            "ones1": np.ones((1, 128), dtype=np.float32),
            "sinT": sinT,
            "maskm": maskm,
        })

    trace = bool(int(os.environ.get("TRNK_TRACE", "0")))
    if trace:
        try:
            import trnprof
            trnprof.install()
        except Exception:
            trace = False
    global LAST_RESULT
    res = run_bass_kernel_spmd(nc, in_maps, list(range(NCORES)), trace=trace)
    LAST_EXEC_TIME_NS = res.exec_time_ns
    LAST_RESULT = res

    # ---- gather / unshard ----
    out = np.zeros((B * T, D), dtype=np.float32)
    for c in range(NCORES):
        out += res.results[c]["out_p"].astype(np.float32)
    out = out.reshape(B, T, D)

    k_all = np.empty((B, G, S, HD), dtype=np.float32)
    v_all = np.empty((B, G, S, HD), dtype=np.float32)
    k_all[:, :, :CACHE] = prev_k
    v_all[:, :, :CACHE] = prev_v
    for c in range(NCORES):
        k_all[:, c, CACHE:] = res.results[c]["knewT"].transpose(0, 2, 1)
        v_all[:, c, CACHE:] = res.results[c]["vnewT"].transpose(0, 2, 1)

    return out, k_all, v_all


# revision 16
# speedup vs baseline: 1.2187x; 1.1495x over previous
"""GQA (ragged_sequence) Trainium2 kernel, 8-core tensor-parallel.

Sharding: heads/KV-groups across the 8 cores (4 query heads + 1 KV group per
core; q_w/o_w split on head axis, k_w/v_w on group axis). Batch stays whole on
every core. o_w is row-split, so each core emits a partial output summed on
host. KV caches shard naturally with the group axis.

Per-core dataflow (all layouts chosen so no on-device transposes are needed
except a tiny PE-transpose for V):
  - qk path in float32r (tf32-ish, ~bf16 speed at N>=512, 16x better accuracy)
  - scores computed transposed: scoresT[s, t] = (k @ qT), softmax sums via
    running bf16 accumulators + a ones-matmul, normalization via a K=1
    broadcast matmul.
  - ctx computed transposed: ctxT[hd, t] = sum_s v[s,hd]^T p[s,t], which feeds
    the o-projection directly (contraction dim on partitions).
"""

import os
import sys
import math

for _p in ("/opt/trn_rl_repo",):
    if _p not in sys.path and os.path.isdir(_p):
        sys.path.insert(0, _p)

import numpy as np
import ml_dtypes
from contextlib import ExitStack

import concourse.bass as bass
import concourse.tile as tile
from concourse import bacc, mybir
from concourse.bass_utils import run_bass_kernel_spmd
from concourse.masks import make_identity

BF16 = ml_dtypes.bfloat16

B, T, D = 4, 512, 4096
H, G, HD = 32, 8, 128
GS = H // G
CACHE = 3584
S = CACHE + T
NCORES = 8
HPC = H // NCORES          # heads per core = 4
KT = D // 128              # 32 contraction tiles
ST = S // 128              # 32 s-tiles
NCT = CACHE // 128         # 28 cache s-tiles
TB = T // 128              # 4 token blocks per batch
OCH = D // 512             # 8 output chunks
SOFTMAX_SCALE = 1.0 / math.sqrt(HD)

_PROGRAM_CACHE = {}
LAST_EXEC_TIME_NS = None
LAST_RESULT = None


def _build_program(act_sets):
    """Build the SPMD Bass program. act_sets: tuple per batch of the s-tile
    indices where the attention mask has any True (mask applied only there)."""
    f32 = mybir.dt.float32
    f32r = mybir.dt.float32r
    bf16 = mybir.dt.bfloat16

    nc = bacc.Bacc("TRN2", target_bir_lowering=False, debug=False,
                   num_devices=NCORES)

    # ---- DRAM I/O (per-core shards) ----
    xT_d = nc.dram_tensor("xT", [D, B * T], f32r, kind="ExternalInput")
    qw_d = nc.dram_tensor("qw", [D, HPC * HD], f32r, kind="ExternalInput")
    kw_d = nc.dram_tensor("kw", [D, HD], f32r, kind="ExternalInput")
    vw_d = nc.dram_tensor("vw", [D, HD], f32r, kind="ExternalInput")
    ow_d = nc.dram_tensor("ow", [HPC * HD, D], bf16, kind="ExternalInput")
    pkT_d = nc.dram_tensor("pkT", [B, HD, CACHE], f32r, kind="ExternalInput")
    pv_d = nc.dram_tensor("pv", [B, CACHE, HD], bf16, kind="ExternalInput")
    cosT_d = nc.dram_tensor("cosT", [B, HD, T], f32, kind="ExternalInput")
    sinT_d = nc.dram_tensor("sinT", [B, HD, T], f32, kind="ExternalInput")
    ones1_d = nc.dram_tensor("ones1", [1, 128], mybir.dt.float32r, kind="ExternalInput")
    n_mask = max(1, sum(len(a) for a in act_sets))
    mask_d = nc.dram_tensor("maskm", [n_mask, 128, 2 * T], bf16, kind="ExternalInput")
    mask_idx = {}
    mi = 0
    for b in range(B):
        for j in act_sets[b]:
            mask_idx[(b, j)] = mi
            mi += 1

    out_d = nc.dram_tensor("out_p", [B * T, D], bf16, kind="ExternalOutput")
    knewT_d = nc.dram_tensor("knewT", [B, HD, T], f32, kind="ExternalOutput")
    vnewT_d = nc.dram_tensor("vnewT", [B, HD, T], f32, kind="ExternalOutput")

    with tile.TileContext(nc) as tc, ExitStack() as ctx:
        P = lambda name, bufs, space="SBUF": ctx.enter_context(
            tc.tile_pool(name=name, bufs=bufs, space=space))

        # NOTE: tile tags default to the tile *name*; each tag gets its own
        # `bufs` slots in the pool. PSUM tags are explicit and budgeted to the
        # 8 banks: qacc(3) + mm(3) + ctxps(1) + small(1).
        p_const = P("const", 1)
        p_kw = P("kw", 1)
        p_vw = P("vw", 1)
        p_ow = P("ow", 1)
        p_xt = P("xt", 4)
        p_qw = P("qws", 4)
        p_tab = P("tab", 2)
        p_mask = P("mask", 2)
        p_kT = P("kT", 1)
        p_vall = P("vall", 1)
        p_qT = P("qT", 2)
        p_p = P("pp", 6)
        p_acc = P("acc", 2)
        p_ctx = P("ctx", 2)
        p_tmp = P("tmp", 2)
        p_stage = P("stage", 2)
        p_bc = P("bc", 2)
        p_recip = P("recip", 1)
        p_out = P("outs", 4)
        ps = P("ps", 1, space="PSUM")

        # ---- constants ----
        ident = p_const.tile([128, 128], bf16)
        make_identity(nc, ident)
        onesb = p_const.tile([128, 1], bf16)
        nc.vector.memset(onesb, 1.0)
        ones1 = p_const.tile([1, 128], f32r)
        nc.sync.dma_start(ones1, ones1_d.ap())

        # ---- resident weights ----
        kw_t = p_kw.tile([128, KT, HD], f32r)
        nc.sync.dma_start(kw_t, kw_d.ap().rearrange("(kt p) n -> p kt n", p=128))
        vw_t = p_vw.tile([128, KT, HD], f32r)
        nc.sync.dma_start(vw_t, vw_d.ap().rearrange("(kt p) n -> p kt n", p=128))
        ow_tiles = []

        # PSUM tags: qacc(3 banks, proj accumulators) + mm(3, scores/o-proj)
        # + ctxps(1) + small(1) = 8 banks. Separate tags let phase A(b+1)
        # overlap attention(b) instead of serializing through shared slots.
        for b in range(B):
            bt = slice(b * T, (b + 1) * T)

            # ======== Phase A: two passes of 3 accumulators ========
            cos_b = p_tab.tile([128, T], f32, name="cosb")
            nc.sync.dma_start(cos_b, cosT_d.ap()[b])
            sin_b = p_tab.tile([128, T], f32, name="sinb")
            nc.sync.dma_start(sin_b, sinT_d.ap()[b])
            kT_b = p_kT.tile([128, S], f32r, name="kTb")
            nc.sync.dma_start(kT_b[:, 0:CACHE], pkT_d.ap()[b])
            v_b = p_vall.tile([128, S], bf16, name="vb")
            nc.sync.dma_start(
                v_b[:, 0:CACHE].rearrange("p (j h) -> p j h", h=HD),
                pv_d.ap()[b].rearrange("(j p) h -> p j h", p=128))

            def rope(dst, src_ap):
                t1 = p_tmp.tile([128, T], f32, name="t1")
                nc.vector.tensor_mul(t1, src_ap, cos_b)
                t2 = p_tmp.tile([128, T], f32, name="t2")
                nc.vector.tensor_mul(t2[0:64, :], src_ap[64:128, :], sin_b[0:64, :])
                nc.vector.tensor_mul(t2[64:128, :], src_ap[0:64, :], sin_b[64:128, :])
                nc.vector.tensor_sub(dst[0:64, :], t1[0:64, :], t2[0:64, :])
                nc.vector.tensor_add(dst[64:128, :], t1[64:128, :], t2[64:128, :])

            qT_b = []
            for hp in range(2):
                ps_a = ps.tile([128, T], f32, name="psa", tag="qacc", bufs=3)
                ps_b2 = ps.tile([128, T], f32, name="psb", tag="qacc", bufs=3)
                ps_c = ps.tile([128, T], f32, name="psc", tag="qacc", bufs=3)
                qcols = slice(hp * 256, hp * 256 + 256)
                kvw = kw_t if hp == 0 else vw_t
                for k in range(KT):
                    xt = p_xt.tile([128, T], f32r, name="xt")
                    nc.sync.dma_start(xt, xT_d.ap()[k * 128:(k + 1) * 128, bt])
                    qwk = p_qw.tile([128, 256], f32r, name="qwk")
                    nc.sync.dma_start(qwk, qw_d.ap()[k * 128:(k + 1) * 128, qcols])
                    first, last = (k == 0), (k == KT - 1)
                    nc.tensor.matmul(ps_a, qwk[:, 0:128], xt, start=first, stop=last)
                    nc.tensor.matmul(ps_b2, qwk[:, 128:256], xt, start=first, stop=last)
                    nc.tensor.matmul(ps_c, kvw[:, k, :], xt, start=first, stop=last)

                for m, srcp in ((2 * hp, ps_a), (2 * hp + 1, ps_b2)):
                    qt = p_qT.tile([128, T], f32r, name=f"qT{m}")
                    rope(qt, srcp)
                    qT_b.append(qt)

                if hp == 0:
                    rope(kT_b[:, CACHE:S], ps_c)
                    nc.sync.dma_start(knewT_d.ap()[b],
                                        kT_b[:, CACHE:S].bitcast(mybir.dt.float32))
                else:
                    vst = p_stage.tile([128, T], f32, name="vst")
                    nc.scalar.copy(vst, ps_c)
                    nc.sync.dma_start(vnewT_d.ap()[b], vst)
                    vT_sb = p_stage.tile([128, T], bf16, name="vTsb")
                    nc.vector.tensor_copy(vT_sb, ps_c)
                    for tb in range(TB):
                        ps_tr = ps.tile([128, 128], bf16, name="pstr",
                                        tag="small", bufs=1)
                        nc.tensor.transpose(ps_tr, vT_sb[:, tb * 128:(tb + 1) * 128],
                                            ident)
                        nc.vector.tensor_copy(
                            v_b[:, CACHE + tb * 128:CACHE + (tb + 1) * 128], ps_tr)

            if b == 0:
                for h in range(HPC):
                    owh = p_ow.tile([128, D], bf16, name=f"ow{h}")
                    nc.sync.dma_start(owh, ow_d.ap()[h * 128:(h + 1) * 128, :])
                    ow_tiles.append(owh)

            # ======== Phase B: attention per local head ========
            ctx_sb = []
            for h in range(HPC):
                accs = [p_acc.tile([128, T], bf16, name=f"a{i}") for i in range(4)]
                ctx_ps = ps.tile([128, T], f32, name="ctxps", tag="ctxps", bufs=1)
                for j in range(ST):
                    sc = ps.tile([128, T], f32, name="sc", tag="mm", bufs=3)
                    nc.tensor.matmul(sc, kT_b[:, j * 128:(j + 1) * 128], qT_b[h],
                                     start=True, stop=True)
                    p_j = p_p.tile([128, T], bf16, name="pj")
                    nc.scalar.activation(p_j, sc, mybir.ActivationFunctionType.Exp,
                                         scale=SOFTMAX_SCALE)
                    if (b, j) in mask_idx:
                        mt = p_mask.tile([128, T], bf16, name="mt")
                        nc.sync.dma_start(mt, mask_d.ap()[mask_idx[(b, j)], :, 0:T])
                        nc.vector.tensor_mul(p_j, p_j, mt)
                    ia = j % 4
                    if j < 4:
                        nc.vector.tensor_copy(accs[ia], p_j)
                    else:
                        nc.vector.tensor_add(accs[ia], accs[ia], p_j)
                    nc.tensor.matmul(ctx_ps, v_b[:, j * 128:(j + 1) * 128], p_j,
                                     start=(j == 0), stop=(j == ST - 1))
                nc.vector.tensor_add(accs[0], accs[0], accs[1])
                nc.vector.tensor_add(accs[2], accs[2], accs[3])
                nc.vector.tensor_add(accs[0], accs[0], accs[2])
                sums_ps = ps.tile([1, T], f32, name="sums", tag="small", bufs=1)
                nc.tensor.matmul(sums_ps, onesb, accs[0], start=True, stop=True)
                recip32 = p_recip.tile([1, T], f32, name="recip32")
                rscr = p_recip.tile([1, T], f32, name="rscr")
                nc.vector.reciprocal_approx_accurate(recip32, sums_ps, rscr)
                recip = p_recip.tile([1, T], f32r, name="recip")
                nc.vector.tensor_copy(recip, recip32)
                bc_ps = ps.tile([128, T], f32, name="bcps", tag="small", bufs=1)
                nc.tensor.matmul(bc_ps, ones1, recip, start=True, stop=True)
                bc_sb = p_bc.tile([128, T], f32, name="bcsb")
                nc.vector.tensor_copy(bc_sb, bc_ps)
                cx = p_ctx.tile([128, T], bf16, name=f"cx{h}")
                nc.vector.tensor_mul(cx, ctx_ps, bc_sb)
                ctx_sb.append(cx)

            # ======== Phase C: partial o-projection ========
            for tb in range(TB):
                for chk in range(OCH):
                    ops_t = ps.tile([128, 512], f32, name="ops", tag="mm", bufs=3)
                    for h in range(HPC):
                        nc.tensor.matmul(
                            ops_t, ctx_sb[h][:, tb * 128:(tb + 1) * 128],
                            ow_tiles[h][:, chk * 512:(chk + 1) * 512],
                            start=(h == 0), stop=(h == HPC - 1))
                    o_sb = p_out.tile([128, 512], bf16, name="osb")
                    nc.vector.tensor_copy(o_sb, ops_t)
                    nc.sync.dma_start(
                        out_d.ap()[b * T + tb * 128:b * T + (tb + 1) * 128,
                                   chk * 512:(chk + 1) * 512], o_sb)

    nc.compile()
    return nc


def kernel(x, mask, cos, sin, start_positions, prev_k, prev_v, q_w, k_w, v_w, o_w):
    global LAST_EXEC_TIME_NS
    x = np.asarray(x, dtype=np.float32)
    mask = np.asarray(mask)
    cos = np.asarray(cos, dtype=np.float32)
    sin = np.asarray(sin, dtype=np.float32)
    start_positions = np.asarray(start_positions)
    prev_k = np.asarray(prev_k, dtype=np.float32)
    prev_v = np.asarray(prev_v, dtype=np.float32)
    q_w = np.asarray(q_w, dtype=np.float32)
    k_w = np.asarray(k_w, dtype=np.float32)
    v_w = np.asarray(v_w, dtype=np.float32)
    o_w = np.asarray(o_w, dtype=np.float32)

    # ---- host-side prep (layout/sharding only) ----
    m = np.asarray(mask[:, 0], dtype=bool)                      # [B, T, S]
    act = m.reshape(B, T, ST, 128).any(axis=(1, 3))             # [B, ST]
    act_sets = tuple(tuple(int(j) for j in np.nonzero(act[b])[0]) for b in range(B))

    key = act_sets
    if key not in _PROGRAM_CACHE:
        _PROGRAM_CACHE[key] = _build_program(act_sets)
    nc = _PROGRAM_CACHE[key]

    xT = np.ascontiguousarray(x.reshape(B * T, D).T)            # [D, B*T] f32
    pos = start_positions.astype(np.int64)[:, None] + np.arange(T)[None, :]
    cosT = np.ascontiguousarray(cos[pos].transpose(0, 2, 1))    # [B, HD, T]
    sinT = np.ascontiguousarray(sin[pos].transpose(0, 2, 1))

    mask_tiles = []
    for b in range(B):
        for j in act_sets[b]:
            t1 = (~m[b, :, j * 128:(j + 1) * 128]).T.astype(BF16)
            mask_tiles.append(np.concatenate([t1, t1], axis=1))
    if mask_tiles:
        maskm = np.ascontiguousarray(np.stack(mask_tiles))
    else:
        maskm = np.zeros((1, 128, 2 * T), dtype=BF16)

    in_maps = []
    for c in range(NCORES):
        hsl = slice(c * HPC * HD, (c + 1) * HPC * HD)
        gsl = slice(c * HD, (c + 1) * HD)
        in_maps.append({
            "xT": xT,
            "qw": np.ascontiguousarray(q_w[:, hsl]),
            "kw": np.ascontiguousarray(k_w[:, gsl]),
            "vw": np.ascontiguousarray(v_w[:, gsl]),
            "ow": np.ascontiguousarray(o_w[hsl, :]).astype(BF16),
            "pkT": np.ascontiguousarray(prev_k[:, c].transpose(0, 2, 1)),
            "pv": np.ascontiguousarray(prev_v[:, c]).astype(BF16),
            "cosT": cosT,
            "ones1": np.ones((1, 128), dtype=np.float32),
            "sinT": sinT,
            "maskm": maskm,
        })

    trace = bool(int(os.environ.get("TRNK_TRACE", "0")))
    if trace:
        try:
            import trnprof
            trnprof.install()
        except Exception:
            trace = False
    global LAST_RESULT
    res = run_bass_kernel_spmd(nc, in_maps, list(range(NCORES)), trace=trace)
    LAST_EXEC_TIME_NS = res.exec_time_ns
    LAST_RESULT = res

    # ---- gather / unshard ----
    out = np.zeros((B * T, D), dtype=np.float32)
    for c in range(NCORES):
        out += res.results[c]["out_p"].astype(np.float32)
    out = out.reshape(B, T, D)

    k_all = np.empty((B, G, S, HD), dtype=np.float32)
    v_all = np.empty((B, G, S, HD), dtype=np.float32)
    k_all[:, :, :CACHE] = prev_k
    v_all[:, :, :CACHE] = prev_v
    for c in range(NCORES):
        k_all[:, c, CACHE:] = res.results[c]["knewT"].transpose(0, 2, 1)
        v_all[:, c, CACHE:] = res.results[c]["vnewT"].transpose(0, 2, 1)

    return out, k_all, v_all
